# revision 49
# baseline (speedup 1.0000x reference)
"""Griffin block (Hawk RG-LRU + GatedMLP) Trainium2 Bass kernel.

Sharding: 8 chunks = 4 batches x 2 time-halves, one per NeuronCore.
Per-core layout is feature-major ([channels, tokens]).

v4 structure (single-residency, seam-free):
  - x arrives bf16 (host-cast): no on-chip casts, no xb spill; phase 2
    re-reads x straight from DRAM into the x1 tiles (in-place residual).
  - rmsnorm commutes through the projections, so the per-token scale
    s = 1/||x|| is folded into the PSUM->SBUF drains.  s itself is a
    vector square -> ones-matmul -> scalar Rsqrt chain (one ACT op).
  - zcb (post-conv activations) stay RESIDENT in SBUF between pass A
    and pass B: no DRAM round trip.
  - the cumulative log-alpha scan (cs) runs inside pass B right after
    la; spills are h and cs only (plus the pass-A gate spill g).
  - phase-2 head: tile-0 reloads ride the idle SYNC queue and their
    exp/mult prep is emitted inside pass B, so the PE flows from the
    pass-B matmuls straight into the carry-free output projection; the
    pairwise carry AllGather overlaps it, and every later tile uses the
    fused carry fixup.
  - spill/reload DMAs move 8-channel groups as single descriptors.
  - engine placement: conv split vector/gpsimd; pass-B elementwise
    spread across vector/gpsimd; ACT work batched per function.
"""

import numpy as np
import ml_dtypes
from contextlib import ExitStack

import concourse.bass as bass
import concourse.bacc as bacc
import concourse.tile as tile
from concourse import mybir
from concourse.bass_utils import run_bass_kernel_spmd

F32 = mybir.dt.float32
BF16 = mybir.dt.bfloat16
AF = mybir.ActivationFunctionType
OP = mybir.AluOpType

D = 1024
NP = 128          # partitions
NCT = D // NP     # channel tiles = 8
KCONV = 4
N_CORES = 8

_BF = ml_dtypes.bfloat16


def build_program(T_core: int, L: int, gelu_approx: bool = False,
                  L2: int | None = None):
    """Emit the SPMD program. T_core tokens per core, token tile L."""
    assert T_core % L == 0
    n_tiles = T_core // L
    if L2 is None:
        L2 = L
    n_tiles2 = T_core // L2
    H2 = 2 * D        # hawk proj width (2048)
    HID = 2 * H2      # gmlp hidden rows (4096): gate2 [0:2048), v [2048:4096)
    GELU = AF.Gelu_apprx_sigmoid if gelu_approx else AF.Gelu

    nc = bacc.Bacc("TRN2", target_bir_lowering=False, debug=False,
                   num_devices=N_CORES)

    # ---- DRAM parameters (per-core data via in_maps) ----
    x_d = nc.dram_tensor("x", [D, 3 + T_core], BF16, kind="ExternalInput")
    wi_d = nc.dram_tensor("wi", [D, H2], BF16, kind="ExternalInput")      # input_w.T (gamma folded)
    wg_d = nc.dram_tensor("wg", [D, H2], BF16, kind="ExternalInput")      # gates_w.T
    wo_d = nc.dram_tensor("wo", [D, D], BF16, kind="ExternalInput")       # output_w.T
    wgr_d = nc.dram_tensor("wgr", [D, HID], BF16, kind="ExternalInput")   # grow_w.T (gamma folded)
    wsh_d = nc.dram_tensor("wsh", [H2, D], BF16, kind="ExternalInput")    # shrink_w.T
    # per-channel params, laid out [partition, ch_tile]
    msp_d = nc.dram_tensor("msp", [NP, NCT], F32, kind="ExternalInput")    # -8*softplus(fb)
    gbf_d = nc.dram_tensor("gbf", [NP, NCT], F32, kind="ExternalInput")    # gates_b[:D]
    gbi_d = nc.dram_tensor("gbi", [NP, NCT], F32, kind="ExternalInput")    # gates_b[D:]
    cw_d = nc.dram_tensor("cw", [NP, KCONV * NCT], F32, kind="ExternalInput")  # conv w taps
    cb_d = nc.dram_tensor("cb", [NP, NCT], F32, kind="ExternalInput")      # conv bias
    cmask_d = nc.dram_tensor("cmask", [NP, 1], F32, kind="ExternalInput")  # 1.0 iff second half

    out_d = nc.dram_tensor("out", [D, T_core], F32, kind="ExternalOutput")

    # ---- internal DRAM scratch ----
    h_d = nc.dram_tensor("h_spill", [D, T_core], BF16)
    cs_d = nc.dram_tensor("cs_spill", [D, T_core], BF16)
    g_d = nc.dram_tensor("g_spill", [D, T_core], BF16)
    carry_loc = nc.dram_tensor("carry_loc", [1, D], BF16)
    carry_gth = nc.dram_tensor("carry_gth", [2, D], BF16)
    warm_loc = nc.dram_tensor("warm_loc", [1, NP], BF16)
    warm_gth = nc.dram_tensor("warm_gth", [2, NP], BF16)

    def blk_in(dram_ap, nblk):
        # DRAM [nblk*NP, C] -> [NP, nblk, C] (channel-block-major view)
        return dram_ap.rearrange("(k p) c -> p k c", k=nblk)

    def blk_sb(tile_ap, nblk):
        # SBUF [NP, nblk*C] -> [NP, nblk, C]
        return tile_ap.rearrange("p (k c) -> p k c", k=nblk)

    with tile.TileContext(nc) as tc, ExitStack() as top:
        # ------- persistent small constants -------
        cpool = top.enter_context(tc.tile_pool(name="consts", bufs=1))
        ones_bf = cpool.tile([NP, NP], BF16, name="ones_bf")
        nc.vector.memset(ones_bf[:], 1.0)
        ones_f = cpool.tile([NP, L], BF16, name="ones_f")
        nc.vector.memset(ones_f[:], 1.0)
        msp_sb = cpool.tile([NP, NCT], F32, name="msp_sb")
        nc.gpsimd.dma_start(msp_sb[:], msp_d.ap()[:, :])
        gbf_sb = cpool.tile([NP, NCT], F32, name="gbf_sb")
        nc.gpsimd.dma_start(gbf_sb[:], gbf_d.ap()[:, :])
        gbi_sb = cpool.tile([NP, NCT], F32, name="gbi_sb")
        nc.gpsimd.dma_start(gbi_sb[:], gbi_d.ap()[:, :])
        cw_sb = cpool.tile([NP, KCONV * NCT], F32, name="cw_sb")
        nc.gpsimd.dma_start(cw_sb[:], cw_d.ap()[:, :])
        cb_sb = cpool.tile([NP, NCT], F32, name="cb_sb")
        nc.gpsimd.dma_start(cb_sb[:], cb_d.ap()[:, :])
        cmask_sb = cpool.tile([NP, 1], F32, name="cmask_sb")
        nc.gpsimd.dma_start(cmask_sb[:], cmask_d.ap()[:, :])
        epsb = cpool.tile([NP, 1], F32, name="epsb")
        nc.vector.memset(epsb[:], 1e-6)
        onepb = cpool.tile([NP, 1], F32, name="onepb")
        nc.vector.memset(onepb[:], 1.0 + 1e-6)
        hlast = cpool.tile([NP, NCT], F32, name="hlast")
        cslast = cpool.tile([NP, NCT], F32, name="cslast")

        # warm up the collective machinery early so the real carry
        # exchange at the pass-B tail is cheap
        nc.gpsimd.dma_start(warm_loc.ap()[0:1, :], ones_bf[0:1, 0:NP])
        nc.gpsimd.collective_compute(
            "AllGather", OP.bypass,
            replica_groups=[[0, 1], [2, 3], [4, 5], [6, 7]],
            ins=[warm_loc.ap()], outs=[warm_gth.ap()])

        pmm2 = top.enter_context(tc.tile_pool(name="pmm2", bufs=2,
                                              space="PSUM"))
        # phase-2 weights whose loads overlap earlier phases
        w2a = top.enter_context(tc.tile_pool(name="w2a", bufs=1,
                                             side="right"))
        wo_sb = w2a.tile([NP, NCT * D], BF16, name="wo_sb")

        # placeholders filled in after pass A (pools opened late to keep
        # pass-A SBUF pressure down)
        hp2 = crp = None
        hr_a = [[None] * NCT for _ in range(n_tiles2)]
        gr_a = [[None] * NCT for _ in range(n_tiles2)]
        cr_a = [[None] * NCT for _ in range(n_tiles2)]
        x1_a = [[None] * NCT for _ in range(n_tiles2)]
        ac_a = [[None] * NCT for _ in range(n_tiles2)]
        u_0 = [None] * NCT
        v_0 = [None] * NCT

        def p2_loads(t):
            # hr/gr ride the sync queue (consumers live on gpsimd and
            # vector), cr/x1 the gpsimd queue (consumers on scalar and
            # vector): no issue stream ever waits on work queued behind
            # it on its own engine
            for i in range(NCT):
                hri = hp2.tile([NP, L2], BF16, name=f"hr_{t}_{i}", tag="hr")
                nc.sync.dma_start(
                    hri[:],
                    h_d.ap()[i * NP:(i + 1) * NP, t * L2:(t + 1) * L2])
                hr_a[t][i] = hri
                gri = hp2.tile([NP, L2], BF16, name=f"gr_{t}_{i}", tag="gr")
                nc.sync.dma_start(
                    gri[:],
                    g_d.ap()[i * NP:(i + 1) * NP, t * L2:(t + 1) * L2])
                gr_a[t][i] = gri
                cri = crp.tile([NP, L2], BF16, name=f"cr_{t}_{i}", tag="cr",
                               bufs=7)
                nc.gpsimd.dma_start(
                    cri[:],
                    cs_d.ap()[i * NP:(i + 1) * NP, t * L2:(t + 1) * L2])
                cr_a[t][i] = cri
                # x1 tile starts as the bf16 residual x; the output
                # projection accumulates into it in place
                x1i = crp.tile([NP, L2], BF16, name=f"x1_{t}_{i}",
                               tag="x1", bufs=10)
                nc.gpsimd.dma_start(
                    x1i[:],
                    x_d.ap()[i * NP:(i + 1) * NP,
                             3 + t * L2:3 + (t + 1) * L2])
                x1_a[t][i] = x1i

        # =========================== PHASE 1 ===========================
        with ExitStack() as p1:
            w1b = p1.enter_context(tc.tile_pool(name="w1b", bufs=1))
            wg_sb = w1b.tile([NP, NCT * H2], BF16, name="wg_sb")
            # zcb stays resident between pass A and pass B
            zres = p1.enter_context(tc.tile_pool(name="zres", bufs=1))
            zcb_res = [zres.tile([NP, NCT * L], BF16, name=f"zcbr_{t}")
                       for t in range(n_tiles)]

            # ------------------------- PASS A -------------------------
            with ExitStack() as pA:
                w1a = pA.enter_context(tc.tile_pool(name="w1a", bufs=1))
                wi_sb = w1a.tile([NP, NCT * H2], BF16, name="wi_sb")
                # z-half first (needed by the halo projection); x tile 0
                # loads on sync in parallel
                for k in range(NCT):
                    [nc.scalar, nc.gpsimd][k % 2].dma_start(
                        wi_sb[:, k * H2 + D:(k + 1) * H2],
                        wi_d.ap()[k * NP:(k + 1) * NP, D:])
                for k in range(NCT):
                    [nc.scalar, nc.gpsimd][k % 2].dma_start(
                        wi_sb[:, k * H2:k * H2 + D],
                        wi_d.ap()[k * NP:(k + 1) * NP, :D])
                nc.scalar.dma_start(blk_sb(wg_sb[:], NCT),
                                    blk_in(wg_d.ap()[:, :], NCT))

                pmm = pA.enter_context(
                    tc.tile_pool(name="pmm", bufs=6, space="PSUM"))
                xbp = pA.enter_context(tc.tile_pool(name="xbp", bufs=2))
                sqp = pA.enter_context(tc.tile_pool(name="sqp", bufs=2))
                sp = pA.enter_context(tc.tile_pool(name="sp", bufs=2))
                zp = pA.enter_context(tc.tile_pool(name="zp", bufs=9))
                zcp = pA.enter_context(tc.tile_pool(name="zcp", bufs=4))
                gp = pA.enter_context(tc.tile_pool(name="gp", bufs=3))
                gbp = pA.enter_context(tc.tile_pool(name="gbp", bufs=2))
                zhp = pA.enter_context(tc.tile_pool(name="zhp", bufs=1))

                # ---- halo: z for the 3 tokens before this chunk ----
                xbh = xbp.tile([NP, 3 * NCT], BF16, name="xbh", tag="xbh",
                               bufs=1)
                nc.sync.dma_start(blk_sb(xbh[:], NCT),
                                  blk_in(x_d.ap()[:, 0:3], NCT))
                xsqh = sqp.tile([NP, 3 * NCT], BF16, name="xsqh",
                                tag="xsqh", bufs=1)
                nc.vector.tensor_tensor(xsqh[:], xbh[:], xbh[:], OP.mult)
                ssqh = pmm.tile([NP, 3], F32, name="ssqh", tag="mm")
                for i in range(NCT):
                    nc.tensor.matmul(ssqh[:], ones_bf[:],
                                     xsqh[:, 3 * i:3 * i + 3],
                                     start=(i == 0), stop=(i == NCT - 1))
                shq = sp.tile([NP, 3], F32, name="shq", tag="sq")
                nc.scalar.activation(shq[:], ssqh[:], AF.Sqrt,
                                     bias=epsb[:, 0:1])
                sh_ = sp.tile([NP, 3], F32, name="sh_", tag="s")
                nc.vector.reciprocal(sh_[:], shq[:])
                zhalo = zhp.tile([NP, 3 * NCT], BF16, name="zhalo",
                                 tag="zhalo")
                for m in range(NCT):
                    ps = pmm.tile([NP, 3], F32, name=f"zh_ps_{m}", tag="mm")
                    for k in range(NCT):
                        lhs = wi_sb[:, k * H2 + D + m * NP:
                                    k * H2 + D + (m + 1) * NP]
                        nc.tensor.matmul(ps[:], lhs,
                                         xbh[:, 3 * k:3 * k + 3],
                                         start=(k == 0), stop=(k == NCT - 1))
                    nc.vector.tensor_tensor(zhalo[:, 3 * m:3 * m + 3], ps[:],
                                            sh_[:], OP.mult)

                z_prev = [None] * NCT
                xb_t = [None, None]
                xb_t[0] = xbp.tile([NP, NCT * L], BF16, name="xb_0",
                                   tag="xb")
                nc.sync.dma_start(blk_sb(xb_t[0][:], NCT),
                                  blk_in(x_d.ap()[:, 3:3 + L], NCT))
                for t in range(n_tiles):
                    xb = xb_t[t % 2]
                    if t + 1 < n_tiles:
                        c0 = 3 + (t + 1) * L
                        xb_t[(t + 1) % 2] = xbp.tile(
                            [NP, NCT * L], BF16, name=f"xb_{t + 1}",
                            tag="xb")
                        nc.sync.dma_start(
                            blk_sb(xb_t[(t + 1) % 2][:], NCT),
                            blk_in(x_d.ap()[:, c0:c0 + L], NCT))
                    if t == 1:  # wo prefetch once the wi queue has drained
                        nc.sync.dma_start(blk_sb(wo_sb[:], NCT),
                                          blk_in(wo_d.ap()[:, :], NCT))

                    # token norm scale s = 1/||x|| (broadcast over parts)
                    xsq = sqp.tile([NP, NCT * L], BF16, name=f"xsq_{t}",
                                   tag="xsq")
                    nc.scalar.activation(xsq[:], xb[:], AF.Square)
                    ssq = pmm.tile([NP, L], F32, name=f"ssq_{t}", tag="mm")
                    for i in range(NCT):
                        nc.tensor.matmul(ssq[:], ones_bf[:],
                                         xsq[:, i * L:(i + 1) * L],
                                         start=(i == 0), stop=(i == NCT - 1))
                    sq_ = sp.tile([NP, L], F32, name=f"sq_{t}", tag="sq")
                    nc.scalar.activation(sq_[:], ssq[:], AF.Sqrt,
                                         bias=epsb[:, 0:1])
                    s = sp.tile([NP, L], F32, name=f"s_{t}", tag="s")
                    nc.vector.reciprocal(s[:], sq_[:])

                    # input proj, z half
                    z_cur = [None] * NCT
                    for m in range(NCT):
                        ps = pmm.tile([NP, L], F32, name=f"u_ps_{t}_{m}",
                                      tag="mm")
                        for k in range(NCT):
                            lhs = wi_sb[:, k * H2 + D + m * NP:
                                        k * H2 + D + (m + 1) * NP]
                            nc.tensor.matmul(ps[:], lhs,
                                             xb[:, k * L:(k + 1) * L],
                                             start=(k == 0),
                                             stop=(k == NCT - 1))
                        zt = zp.tile([NP, L + 3], BF16, name=f"z_{t}_{m}",
                                     tag="z")
                        nc.vector.tensor_tensor(zt[:, 3:3 + L], ps[:], s[:],
                                                OP.mult)
                        if t == 0:
                            nc.gpsimd.tensor_copy(zt[:, 0:3],
                                                  zhalo[:, 3 * m:3 * m + 3])
                        else:
                            nc.gpsimd.tensor_copy(zt[:, 0:3],
                                                  z_prev[m][:, L:L + 3])
                        z_cur[m] = zt
                    z_prev = z_cur

                    # depthwise causal conv (taps 0-1 vector, 2-3 gpsimd)
                    for i in range(NCT):
                        zci = zcp.tile([NP, L], BF16, name=f"zc_{t}_{i}",
                                       tag="zc")
                        nc.vector.tensor_scalar(
                            zci[:], z_cur[i][:, 0:L],
                            cw_sb[:, 0 * NCT + i:0 * NCT + i + 1],
                            cb_sb[:, i:i + 1], op0=OP.mult, op1=OP.add)
                        nc.vector.scalar_tensor_tensor(
                            zci[:], z_cur[i][:, 1:1 + L],
                            cw_sb[:, 1 * NCT + i:1 * NCT + i + 1],
                            zci[:], op0=OP.mult, op1=OP.add)
                        nc.vector.scalar_tensor_tensor(
                            zci[:], z_cur[i][:, 2:2 + L],
                            cw_sb[:, 2 * NCT + i:2 * NCT + i + 1],
                            zci[:], op0=OP.mult, op1=OP.add)
                        nc.vector.scalar_tensor_tensor(
                            zcb_res[t][:, i * L:(i + 1) * L],
                            z_cur[i][:, 3:3 + L],
                            cw_sb[:, 3 * NCT + i:3 * NCT + i + 1],
                            zci[:], op0=OP.mult, op1=OP.add)

                    # gate half of the input proj -> gelu -> spill
                    gbig = gbp.tile([NP, NCT * L], BF16, name=f"gbig_{t}",
                                    tag="gbig")
                    for m in range(NCT):
                        ps = pmm.tile([NP, L], F32, name=f"g_ps_{t}_{m}",
                                      tag="mm")
                        for k in range(NCT):
                            lhs = wi_sb[:, k * H2 + m * NP:
                                        k * H2 + (m + 1) * NP]
                            nc.tensor.matmul(ps[:], lhs,
                                             xb[:, k * L:(k + 1) * L],
                                             start=(k == 0),
                                             stop=(k == NCT - 1))
                        g1 = gp.tile([NP, L], BF16, name=f"g1_{t}_{m}",
                                     tag="g1")
                        nc.vector.tensor_tensor(g1[:], ps[:], s[:], OP.mult)
                        if gelu_approx:
                            gs = gp.tile([NP, L], F32, name=f"gs_{t}_{m}",
                                         tag="gs", bufs=2)
                            nc.scalar.activation(gs[:], g1[:], AF.Sigmoid,
                                                 scale=1.702)
                            nc.vector.tensor_tensor(
                                gbig[:, m * L:(m + 1) * L], g1[:], gs[:],
                                OP.mult)
                        else:
                            nc.scalar.activation(gbig[:, m * L:(m + 1) * L],
                                                 g1[:], GELU)
                    nc.sync.dma_start(
                        blk_in(g_d.ap()[:, t * L:(t + 1) * L], NCT),
                        blk_sb(gbig[:], NCT))

            # ------------------- PASS B (wi freed) --------------------
            hp2 = top.enter_context(tc.tile_pool(name="hp2", bufs=7,
                                                 side="right"))
            crp = top.enter_context(tc.tile_pool(name="crp", bufs=8,
                                                 side="right"))

            with ExitStack() as pB:
                pmmb = pB.enter_context(
                    tc.tile_pool(name="pmmb", bufs=6, space="PSUM"))
                sfp = pB.enter_context(tc.tile_pool(name="sfp", bufs=8))
                sip = pB.enter_context(tc.tile_pool(name="sip", bufs=8))
                ap_ = pB.enter_context(tc.tile_pool(name="ap", bufs=3))
                a2p = pB.enter_context(tc.tile_pool(name="a2p", bufs=2))
                bp = pB.enter_context(tc.tile_pool(name="bp", bufs=2))
                xsp = pB.enter_context(tc.tile_pool(name="xsp", bufs=2))
                lap = pB.enter_context(tc.tile_pool(name="lap", bufs=2))
                hbp = pB.enter_context(tc.tile_pool(name="hbp", bufs=1))
                csp = pB.enter_context(tc.tile_pool(name="csp", bufs=1))
                # per-channel -8*softplus(fb) broadcast along tokens (lets
                # the la multiply run on gpsimd, which cannot take scalar
                # pointers)
                mspp = pB.enter_context(tc.tile_pool(name="mspp", bufs=1))
                mspb = mspp.tile([NP, NCT * L], BF16, name="mspb")
                for i in range(NCT):
                    nc.vector.tensor_scalar(mspb[:, i * L:(i + 1) * L],
                                            ones_f[:, 0:L],
                                            msp_sb[:, i:i + 1],
                                            None, op0=OP.mult)
                for t in range(n_tiles):
                    zcb = zcb_res[t]
                    sf = [None] * NCT
                    si = [None] * NCT
                    for i in range(NCT):
                        psf = pmmb.tile([NP, L], F32, name=f"f_ps_{t}_{i}",
                                        tag="mmb")
                        for k in range(NCT):
                            lhs = wg_sb[:, k * H2 + i * NP:
                                        k * H2 + (i + 1) * NP]
                            nc.tensor.matmul(psf[:], lhs,
                                             zcb[:, k * L:(k + 1) * L],
                                             start=(k == 0),
                                             stop=(k == NCT - 1))
                        sfi = sfp.tile([NP, L], BF16, name=f"sf_{t}_{i}",
                                       tag="sf")
                        nc.scalar.activation(sfi[:], psf[:], AF.Sigmoid,
                                             bias=gbf_sb[:, i:i + 1])
                        sf[i] = sfi
                        psi = pmmb.tile([NP, L], F32, name=f"i_ps_{t}_{i}",
                                        tag="mmb")
                        for k in range(NCT):
                            lhs = wg_sb[:, k * H2 + D + i * NP:
                                        k * H2 + D + (i + 1) * NP]
                            nc.tensor.matmul(psi[:], lhs,
                                             zcb[:, k * L:(k + 1) * L],
                                             start=(k == 0),
                                             stop=(k == NCT - 1))
                        sii = sip.tile([NP, L], BF16, name=f"si_{t}_{i}",
                                       tag="si")
                        nc.scalar.activation(sii[:], psi[:], AF.Sigmoid,
                                             bias=gbi_sb[:, i:i + 1])
                        si[i] = sii

                    alpha = [None] * NCT
                    for i in range(NCT):
                        al = ap_.tile([NP, L], F32, name=f"al_{t}_{i}",
                                      tag="alpha")
                        nc.scalar.activation(al[:], sf[i][:], AF.Exp,
                                             scale=msp_sb[:, i:i + 1])
                        alpha[i] = al
                    beta = [None] * NCT
                    for i in range(NCT):
                        # a2 = alpha^2 <= 1 exactly: 1+1e-6-a2 > 0 always
                        a2i = a2p.tile([NP, L], F32, name=f"a2_{t}_{i}",
                                       tag="a2")
                        nc.gpsimd.tensor_tensor(a2i[:], alpha[i][:],
                                                alpha[i][:], OP.mult)
                        be = bp.tile([NP, L], BF16, name=f"be_{t}_{i}",
                                     tag="beta")
                        nc.scalar.activation(be[:], a2i[:], AF.Sqrt,
                                             scale=-1.0, bias=onepb[:, 0:1])
                        beta[i] = be

                    hbig = hbp.tile([NP, NCT * L], BF16, name=f"hbig_{t}",
                                    tag="hbig")
                    csbig = csp.tile([NP, NCT * L], BF16, name=f"csbig_{t}",
                                     tag="csbig")
                    # h scans first: they carry the cross-tile recurrence
                    # and (at t=3) gate the carry exchange
                    for i in range(NCT):
                        xs = xsp.tile([NP, L], BF16, name=f"xs_{t}_{i}",
                                      tag="xs")
                        nc.gpsimd.tensor_tensor(xs[:], si[i][:],
                                                zcb[:, i * L:(i + 1) * L],
                                                OP.mult)
                        nc.vector.tensor_tensor(xs[:], xs[:], beta[i][:],
                                                OP.mult)
                        h_init = 0.0 if t == 0 else hlast[:, i:i + 1]
                        nc.vector.tensor_tensor_scan(
                            hbig[:, i * L:(i + 1) * L], alpha[i][:], xs[:],
                            h_init, op0=OP.mult, op1=OP.add)
                        if t < n_tiles - 1:
                            nc.gpsimd.tensor_copy(
                                hlast[:, i:i + 1],
                                hbig[:, (i + 1) * L - 1:(i + 1) * L])
                    if t == n_tiles - 1:
                        # last-token h of every channel in one strided DMA
                        with tc.high_priority():
                            nc.sync.dma_start(
                                carry_loc.ap()[0:1, :].rearrange(
                                    "a (i p) -> p i a", i=NCT),
                                hbig[:, L - 1:NCT * L:L])
                    nc.sync.dma_start(
                        blk_in(h_d.ap()[:, t * L:(t + 1) * L], NCT),
                        blk_sb(hbig[:], NCT))
                    for i in range(NCT):
                        la = lap.tile([NP, L], BF16, name=f"la_{t}_{i}",
                                      tag="la")
                        nc.gpsimd.tensor_tensor(la[:], sf[i][:],
                                                mspb[:, i * L:(i + 1) * L],
                                                OP.mult)
                        c_init = 0.0 if t == 0 else cslast[:, i:i + 1]
                        nc.vector.tensor_tensor_scan(
                            csbig[:, i * L:(i + 1) * L], ones_f[:, 0:L],
                            la[:], c_init, op0=OP.mult, op1=OP.add)
                        if t < n_tiles - 1:
                            nc.gpsimd.tensor_copy(
                                cslast[:, i:i + 1],
                                csbig[:, (i + 1) * L - 1:(i + 1) * L])
                    nc.sync.dma_start(
                        blk_in(cs_d.ap()[:, t * L:(t + 1) * L], NCT),
                        blk_sb(csbig[:], NCT))

                    # phase-2 head (tile 0) rides the pass-B tail
                    if t == 1:
                        p2_loads(0)
                    elif t == 2:
                        for i in range(NCT):
                            aci = crp.tile([NP, L2], BF16, name=f"ac_0_{i}",
                                           tag="ac")
                            nc.scalar.activation(aci[:], cr_a[0][i][:],
                                                 AF.Exp)
                            ac_a[0][i] = aci
                    elif t == 3:
                        for i in range(NCT):
                            ui = crp.tile([NP, L2], BF16, name=f"u_0_{i}",
                                          tag="gh", bufs=8)
                            nc.gpsimd.tensor_tensor(ui[:], gr_a[0][i][:],
                                                    hr_a[0][i][:], OP.mult)
                            u_0[i] = ui
                            vi = crp.tile([NP, L2], BF16, name=f"v_0_{i}",
                                          tag="v")
                            nc.gpsimd.tensor_tensor(vi[:], gr_a[0][i][:],
                                                    ac_a[0][i][:], OP.mult)
                            v_0[i] = vi

                # ---- pairwise carry exchange ----
                with tc.high_priority():
                    nc.gpsimd.collective_compute(
                        "AllGather", OP.bypass,
                        replica_groups=[[0, 1], [2, 3], [4, 5], [6, 7]],
                        ins=[carry_loc.ap()], outs=[carry_gth.ap()])

        # =========================== PHASE 2 ===========================
        # stream the gmlp weights now that wi/wg/zcb have freed
        w2c = top.enter_context(tc.tile_pool(name="w2c", bufs=1,
                                             side="right"))
        wgr_lo = w2c.tile([NP, (NCT // 2) * HID], BF16, name="wgr_lo")
        wgr_hi = w2c.tile([NP, (NCT // 2) * HID], BF16, name="wgr_hi")
        wsh_sb = w2c.tile([NP, 2 * NCT * D], BF16, name="wsh_sb")
        wq = [nc.scalar, nc.sync]
        for k in range(NCT // 2):
            wq[k % 2].dma_start(wgr_lo[:, k * HID:(k + 1) * HID],
                                wgr_d.ap()[k * NP:(k + 1) * NP, :])
        for k in range(NCT // 2):
            wq[k % 2].dma_start(
                wgr_hi[:, k * HID:(k + 1) * HID],
                wgr_d.ap()[(k + NCT // 2) * NP:(k + NCT // 2 + 1) * NP, :])
        nc.scalar.dma_start(blk_sb(wsh_sb[:, 0:NCT * D], NCT),
                            blk_in(wsh_d.ap()[0:D, :], NCT))
        nc.scalar.dma_start(blk_sb(wsh_sb[:, NCT * D:], NCT),
                            blk_in(wsh_d.ap()[D:, :], NCT))

        with ExitStack() as p2:
            hfp = p2.enter_context(tc.tile_pool(name="hfp", bufs=2))
            sq2p = p2.enter_context(tc.tile_pool(name="sq2p", bufs=1))
            s2p = p2.enter_context(tc.tile_pool(name="s2p", bufs=2))
            t2p = p2.enter_context(tc.tile_pool(name="t2p", bufs=2))
            t2gp = p2.enter_context(tc.tile_pool(name="t2gp", bufs=2))
            gvp = p2.enter_context(tc.tile_pool(name="gvp", bufs=16))
            op_ = p2.enter_context(tc.tile_pool(name="op", bufs=2))
            pgro = p2.enter_context(
                tc.tile_pool(name="pgro", bufs=5, space="PSUM"))
            pssq2 = p2.enter_context(
                tc.tile_pool(name="pssq2", bufs=1, space="PSUM"))

            p2_loads(1)

            cg = cpool.tile([NP, NCT], BF16, name="cg")
            nc.scalar.dma_start(
                cg[:],
                carry_gth.ap()[0:1, :].rearrange("a (i p) -> p (i a)",
                                                 i=NCT))

            # carry-free half Wo@(g*h) for tile 0 flows straight out
            # of pass B and hides the collective latency
            for t, u_t in ((0, u_0),):
                x1 = x1_a[t]
                for m in range(NCT):
                    ps = pmm2.tile([NP, L2], F32, name=f"u_ps_{t}_{m}",
                                   tag="mm2")
                    for k in range(NCT):
                        lhs = wo_sb[:, k * D + m * NP:
                                    k * D + (m + 1) * NP]
                        nc.tensor.matmul(ps[:], lhs, u_t[k][:],
                                         start=(k == 0),
                                         stop=(k == NCT - 1))
                    nc.vector.tensor_tensor(x1[m][:], ps[:], x1[m][:],
                                            OP.add)

            # the carry path must not be scheduled ahead of the
            # carry-free projections: the static scheduler models the
            # collective as fast, so hold these back explicitly
            with tc.tile_wait_until(0.40):
                cmd = cpool.tile([NP, 1], F32, name="cmd")
                nc.vector.tensor_tensor(cmd[:], cmask_sb[:, 0:1],
                                        x1_a[0][NCT - 1][:, 0:1], OP.bypass)
                carrym = cpool.tile([NP, NCT], F32, name="carrym")
                nc.vector.tensor_scalar(carrym[:], cg[:], cmd[:, 0:1], None,
                                        op0=OP.mult)

                for t, v_t in ((0, v_0),):
                    x1 = x1_a[t]
                    for i in range(NCT):
                        nc.vector.tensor_scalar(v_t[i][:], v_t[i][:],
                                                carrym[:, i:i + 1], None,
                                                op0=OP.mult)
                    for m in range(NCT):
                        ps = pmm2.tile([NP, L2], F32,
                                       name=f"v_ps2_{t}_{m}", tag="mm2")
                        for k in range(NCT):
                            lhs = wo_sb[:, k * D + m * NP:
                                        k * D + (m + 1) * NP]
                            nc.tensor.matmul(ps[:], lhs, v_t[k][:],
                                             start=(k == 0),
                                             stop=(k == NCT - 1))
                        nc.vector.tensor_tensor(x1[m][:], ps[:], x1[m][:],
                                                OP.add)

            for t in range(n_tiles2):
                hr, gr, cr, x1 = hr_a[t], gr_a[t], cr_a[t], x1_a[t]
                if t >= 1:
                    acs = [None] * NCT
                    for i in range(NCT):
                        aci = crp.tile([NP, L2], BF16, name=f"ac_{t}_{i}",
                                       tag="ac")
                        nc.scalar.activation(aci[:], cr[i][:], AF.Exp)
                        acs[i] = aci
                    gh = [None] * NCT
                    for i in range(NCT):
                        hfi = hfp.tile([NP, L2], BF16, name=f"hf_{t}_{i}",
                                       tag="hf")
                        nc.vector.scalar_tensor_tensor(hfi[:], acs[i][:],
                                                       carrym[:, i:i + 1],
                                                       hr[i][:],
                                                       op0=OP.mult,
                                                       op1=OP.add)
                        ghi = crp.tile([NP, L2], BF16, name=f"gh_{t}_{i}",
                                       tag="gh", bufs=8)
                        nc.vector.tensor_tensor(ghi[:], gr[i][:], hfi[:],
                                                OP.mult)
                        gh[i] = ghi
                    for m in range(NCT):
                        ps = pmm2.tile([NP, L2], F32, name=f"o_ps_{t}_{m}",
                                       tag="mm2")
                        for k in range(NCT):
                            lhs = wo_sb[:, k * D + m * NP:
                                        k * D + (m + 1) * NP]
                            nc.tensor.matmul(ps[:], lhs, gh[k][:],
                                             start=(k == 0),
                                             stop=(k == NCT - 1))
                        nc.vector.tensor_tensor(x1[m][:], ps[:], x1[m][:],
                                                OP.add)

                # prefetch reloads for tile t+2 (t+1 already in flight)
                if t + 2 < n_tiles2:
                    p2_loads(t + 2)

                # x1 norm scale (broadcast), applied at drains
                ssq2 = pssq2.tile([NP, L2], F32, name=f"ssq2_{t}",
                                  tag="ssq2")
                for i in range(NCT):
                    xsq = sq2p.tile([NP, L2], BF16, name=f"x1sq_{t}_{i}",
                                    tag="x1sq")
                    nc.vector.tensor_tensor(xsq[:], x1[i][:], x1[i][:],
                                            OP.mult)
                    nc.tensor.matmul(ssq2[:], ones_bf[:], xsq[:],
                                     start=(i == 0), stop=(i == NCT - 1))
                s2q = s2p.tile([NP, L2], F32, name=f"s2q_{t}", tag="s2q",
                               bufs=1)
                nc.scalar.activation(s2q[:], ssq2[:], AF.Sqrt,
                                     bias=epsb[:, 0:1])
                s2 = s2p.tile([NP, L2], F32, name=f"s2_{t}", tag="s2")
                nc.vector.reciprocal(s2[:], s2q[:])

                # grow proj: gate2 rows [0:2D), v rows [2D:4D)
                gv = [None] * (2 * NCT)
                for hm in range(2 * NCT):
                    psg = pgro.tile([NP, L2], F32, name=f"g2_ps_{t}_{hm}",
                                    tag="mm2g")
                    for k in range(NCT):
                        wsb = wgr_lo if k < NCT // 2 else wgr_hi
                        kk = k % (NCT // 2)
                        lhs = wsb[:, kk * HID + hm * NP:
                                  kk * HID + (hm + 1) * NP]
                        nc.tensor.matmul(psg[:], lhs, x1[k][:],
                                         start=(k == 0), stop=(k == NCT - 1))
                    psv = pgro.tile([NP, L2], F32, name=f"v_ps_{t}_{hm}",
                                    tag="mm2g")
                    for k in range(NCT):
                        wsb = wgr_lo if k < NCT // 2 else wgr_hi
                        kk = k % (NCT // 2)
                        lhs = wsb[:, kk * HID + H2 + hm * NP:
                                  kk * HID + H2 + (hm + 1) * NP]
                        nc.tensor.matmul(psv[:], lhs, x1[k][:],
                                         start=(k == 0), stop=(k == NCT - 1))
                    t2 = t2p.tile([NP, L2], BF16, name=f"t2_{t}_{hm}",
                                  tag="t2")
                    nc.vector.tensor_tensor(t2[:], psg[:], s2[:], OP.mult)
                    t2g = t2gp.tile([NP, L2], BF16, name=f"t2g_{t}_{hm}",
                                    tag="t2g")
                    if gelu_approx:
                        sg2 = t2gp.tile([NP, L2], F32, name=f"sg2_{t}_{hm}",
                                        tag="sg2")
                        nc.scalar.activation(sg2[:], t2[:], AF.Sigmoid,
                                             scale=1.702)
                        nc.vector.tensor_tensor(t2g[:], t2[:], sg2[:],
                                                OP.mult)
                    else:
                        nc.scalar.activation(t2g[:], t2[:], GELU)
                    gvi = gvp.tile([NP, L2], BF16, name=f"gv_{t}_{hm}",
                                   tag="gv")
                    nc.vector.tensor_tensor(gvi[:], t2g[:], psv[:], OP.mult)
                    gv[hm] = gvi

                # shrink proj (x s2) + residual -> out
                for m in range(NCT):
                    ps = pmm2.tile([NP, L2], F32, name=f"s_ps_{t}_{m}",
                                   tag="mm2")
                    for k in range(2 * NCT):
                        lhs = wsh_sb[:, k * D + m * NP: k * D + (m + 1) * NP]
                        nc.tensor.matmul(ps[:], lhs, gv[k][:],
                                         start=(k == 0),
                                         stop=(k == 2 * NCT - 1))
                    om = op_.tile([NP, L2], F32, name=f"out_{t}_{m}",
                                  tag="out")
                    nc.vector.tensor_tensor(om[:], ps[:], s2[:], OP.mult)
                    nc.vector.tensor_tensor(om[:], om[:], x1[m][:], OP.add)
                    nc.sync.dma_start(
                        out_d.ap()[m * NP:(m + 1) * NP, t * L2:(t + 1) * L2],
                        om[:])

    nc.compile()
    return nc


def host_prepare(inputs, T_core, n_cores=N_CORES):
    """Build per-core in_maps from full inputs."""
    x = np.asarray(inputs["x"], np.float32)            # [B, T, D]
    B, T, _ = x.shape
    halves = n_cores // B
    assert T == halves * T_core

    gam1 = np.asarray(inputs["hawk_norm_gamma"], np.float32)
    gam2 = np.asarray(inputs["gmlp_norm_gamma"], np.float32)
    scale1 = gam1 * np.sqrt(D)
    scale2 = gam2 * np.sqrt(D)

    wi = (np.asarray(inputs["input_w"], np.float32) * scale1[None, :]).T
    wg = np.asarray(inputs["gates_w"], np.float32).T
    wo = np.asarray(inputs["output_w"], np.float32).T
    wgr = (np.asarray(inputs["grow_w"], np.float32) * scale2[None, :]).T
    wsh = np.asarray(inputs["shrink_w"], np.float32).T

    fb = np.asarray(inputs["forget_base"], np.float64)
    msp = (-8.0 * np.log1p(np.exp(fb))).astype(np.float32)

    def chan_layout(v):  # [D] -> [128, 8] with [p, i] = v[128*i + p]
        return np.ascontiguousarray(v.reshape(NCT, NP).T)

    gb = np.asarray(inputs["gates_b"], np.float32)
    cw = np.asarray(inputs["conv_w"], np.float32)[:, 0, :]   # [D, K]
    cb = np.asarray(inputs["conv_b"], np.float32)

    shared = {
        "wi": wi.astype(_BF), "wg": wg.astype(_BF), "wo": wo.astype(_BF),
        "wgr": wgr.astype(_BF), "wsh": wsh.astype(_BF),
        "msp": chan_layout(msp),
        "gbf": chan_layout(gb[:D]), "gbi": chan_layout(gb[D:]),
        "cw": np.concatenate([chan_layout(cw[:, k]) for k in range(KCONV)],
                             axis=1),
        "cb": chan_layout(cb),
    }
    in_maps = []
    for core in range(n_cores):
        b, h = core // halves, core % halves
        xf = np.zeros((D, 3 + T_core), np.float32)
        xf[:, 3:] = x[b, h * T_core:(h + 1) * T_core, :].T
        if h > 0:
            xf[:, 0:3] = x[b, h * T_core - 3:h * T_core, :].T
        m = dict(shared)
        m["x"] = xf.astype(_BF)
        m["cmask"] = np.full((NP, 1), 1.0 if h > 0 else 0.0, np.float32)
        in_maps.append(m)
    return in_maps


def assemble_output(results, B, T, T_core, n_cores=N_CORES):
    halves = n_cores // B
    out = np.empty((B, T, D), np.float32)
    for core in range(n_cores):
        b, h = core // halves, core % halves
        out[b, h * T_core:(h + 1) * T_core, :] = results[core]["out"].T
    return out


_PROG_CACHE = {}


def kernel(**inputs) -> np.ndarray:
    x = np.asarray(inputs["x"])
    B, T, _ = x.shape
    T_core = T * B // N_CORES
    L = 512 if T_core % 512 == 0 else T_core // 4
    key = (T_core, L)
    if key not in _PROG_CACHE:
        _PROG_CACHE[key] = build_program(T_core, L)
    nc = _PROG_CACHE[key]
    in_maps = host_prepare(inputs, T_core)
    res = run_bass_kernel_spmd(nc, in_maps, list(range(N_CORES)))
    return assemble_output(res.results, B, T, T_core)


# revision 50
# speedup vs baseline: 1.0267x; 1.0267x over previous
"""Griffin block (Hawk RG-LRU + GatedMLP) Trainium2 Bass kernel.

Sharding: 8 chunks = 4 batches x 2 time-halves, one per NeuronCore.
Per-core layout is feature-major ([channels, tokens]).

v4 structure (single-residency, seam-free):
  - x arrives bf16 (host-cast): no on-chip casts, no xb spill; phase 2
    re-reads x straight from DRAM into the x1 tiles (in-place residual).
  - rmsnorm commutes through the projections, so the per-token scale
    s = 1/||x|| is folded into the PSUM->SBUF drains.  s itself is a
    vector square -> ones-matmul -> scalar Rsqrt chain (one ACT op).
  - zcb (post-conv activations) stay RESIDENT in SBUF between pass A
    and pass B: no DRAM round trip.
  - the cumulative log-alpha scan (cs) runs inside pass B right after
    la; spills are h and cs only (plus the pass-A gate spill g).
  - phase-2 head: tile-0 reloads ride the idle SYNC queue and their
    exp/mult prep is emitted inside pass B, so the PE flows from the
    pass-B matmuls straight into the carry-free output projection; the
    pairwise carry AllGather overlaps it, and every later tile uses the
    fused carry fixup.
  - spill/reload DMAs move 8-channel groups as single descriptors.
  - engine placement: conv split vector/gpsimd; pass-B elementwise
    spread across vector/gpsimd; ACT work batched per function.
"""

import numpy as np
import ml_dtypes
from contextlib import ExitStack

import concourse.bass as bass
import concourse.bacc as bacc
import concourse.tile as tile
from concourse import mybir
from concourse.bass_utils import run_bass_kernel_spmd

F32 = mybir.dt.float32
BF16 = mybir.dt.bfloat16
AF = mybir.ActivationFunctionType
OP = mybir.AluOpType

D = 1024
NP = 128          # partitions
NCT = D // NP     # channel tiles = 8
KCONV = 4
N_CORES = 8

_BF = ml_dtypes.bfloat16


def build_program(T_core: int, L: int, gelu_approx: bool = False,
                  L2: int | None = None):
    """Emit the SPMD program. T_core tokens per core, token tile L."""
    assert T_core % L == 0
    n_tiles = T_core // L
    if L2 is None:
        L2 = L
    n_tiles2 = T_core // L2
    H2 = 2 * D        # hawk proj width (2048)
    HID = 2 * H2      # gmlp hidden rows (4096): gate2 [0:2048), v [2048:4096)
    GELU = AF.Gelu_apprx_sigmoid if gelu_approx else AF.Gelu

    nc = bacc.Bacc("TRN2", target_bir_lowering=False, debug=False,
                   num_devices=N_CORES)

    # ---- DRAM parameters (per-core data via in_maps) ----
    x_d = nc.dram_tensor("x", [D, 3 + T_core], BF16, kind="ExternalInput")
    wi_d = nc.dram_tensor("wi", [D, H2], BF16, kind="ExternalInput")      # input_w.T (gamma folded)
    wg_d = nc.dram_tensor("wg", [D, H2], BF16, kind="ExternalInput")      # gates_w.T
    wo_d = nc.dram_tensor("wo", [D, D], BF16, kind="ExternalInput")       # output_w.T
    wgr_d = nc.dram_tensor("wgr", [D, HID], BF16, kind="ExternalInput")   # grow_w.T (gamma folded)
    wsh_d = nc.dram_tensor("wsh", [H2, D], BF16, kind="ExternalInput")    # shrink_w.T
    # per-channel params, laid out [partition, ch_tile]
    msp_d = nc.dram_tensor("msp", [NP, NCT], F32, kind="ExternalInput")    # -8*softplus(fb)
    gbf_d = nc.dram_tensor("gbf", [NP, NCT], F32, kind="ExternalInput")    # gates_b[:D]
    gbi_d = nc.dram_tensor("gbi", [NP, NCT], F32, kind="ExternalInput")    # gates_b[D:]
    cw_d = nc.dram_tensor("cw", [NP, KCONV * NCT], F32, kind="ExternalInput")  # conv w taps
    cb_d = nc.dram_tensor("cb", [NP, NCT], F32, kind="ExternalInput")      # conv bias
    cmask_d = nc.dram_tensor("cmask", [NP, 1], F32, kind="ExternalInput")  # 1.0 iff second half

    out_d = nc.dram_tensor("out", [D, T_core], F32, kind="ExternalOutput")

    # ---- internal DRAM scratch ----
    h_d = nc.dram_tensor("h_spill", [D, T_core], BF16)
    cs_d = nc.dram_tensor("cs_spill", [D, T_core], BF16)
    g_d = nc.dram_tensor("g_spill", [D, T_core], BF16)
    carry_loc = nc.dram_tensor("carry_loc", [1, D], BF16)
    carry_gth = nc.dram_tensor("carry_gth", [2, D], BF16)
    warm_loc = nc.dram_tensor("warm_loc", [1, NP], BF16)
    warm_gth = nc.dram_tensor("warm_gth", [2, NP], BF16)

    def blk_in(dram_ap, nblk):
        # DRAM [nblk*NP, C] -> [NP, nblk, C] (channel-block-major view)
        return dram_ap.rearrange("(k p) c -> p k c", k=nblk)

    def blk_sb(tile_ap, nblk):
        # SBUF [NP, nblk*C] -> [NP, nblk, C]
        return tile_ap.rearrange("p (k c) -> p k c", k=nblk)

    with tile.TileContext(nc) as tc, ExitStack() as top:
        # ------- persistent small constants -------
        cpool = top.enter_context(tc.tile_pool(name="consts", bufs=1))
        ones_bf = cpool.tile([NP, NP], BF16, name="ones_bf")
        nc.vector.memset(ones_bf[:], 1.0)
        ones_f = cpool.tile([NP, L], BF16, name="ones_f")
        nc.vector.memset(ones_f[:], 1.0)
        msp_sb = cpool.tile([NP, NCT], F32, name="msp_sb")
        nc.gpsimd.dma_start(msp_sb[:], msp_d.ap()[:, :])
        gbf_sb = cpool.tile([NP, NCT], F32, name="gbf_sb")
        nc.gpsimd.dma_start(gbf_sb[:], gbf_d.ap()[:, :])
        gbi_sb = cpool.tile([NP, NCT], F32, name="gbi_sb")
        nc.gpsimd.dma_start(gbi_sb[:], gbi_d.ap()[:, :])
        cw_sb = cpool.tile([NP, KCONV * NCT], F32, name="cw_sb")
        nc.gpsimd.dma_start(cw_sb[:], cw_d.ap()[:, :])
        cb_sb = cpool.tile([NP, NCT], F32, name="cb_sb")
        nc.gpsimd.dma_start(cb_sb[:], cb_d.ap()[:, :])
        cmask_sb = cpool.tile([NP, 1], F32, name="cmask_sb")
        nc.gpsimd.dma_start(cmask_sb[:], cmask_d.ap()[:, :])
        epsb = cpool.tile([NP, 1], F32, name="epsb")
        nc.vector.memset(epsb[:], 1e-6)
        onepb = cpool.tile([NP, 1], F32, name="onepb")
        nc.vector.memset(onepb[:], 1.0 + 1e-6)
        hlast = cpool.tile([NP, NCT], F32, name="hlast")
        cslast = cpool.tile([NP, NCT], F32, name="cslast")

        # warm up the collective machinery early so the real carry
        # exchange at the pass-B tail is cheap
        nc.gpsimd.dma_start(warm_loc.ap()[0:1, :], ones_bf[0:1, 0:NP])
        nc.gpsimd.collective_compute(
            "AllGather", OP.bypass,
            replica_groups=[[0, 1], [2, 3], [4, 5], [6, 7]],
            ins=[warm_loc.ap()], outs=[warm_gth.ap()])

        pmm2 = top.enter_context(tc.tile_pool(name="pmm2", bufs=2,
                                              space="PSUM"))
        # phase-2 weights whose loads overlap earlier phases
        w2a = top.enter_context(tc.tile_pool(name="w2a", bufs=1,
                                             side="right"))
        wo_sb = w2a.tile([NP, NCT * D], BF16, name="wo_sb")

        # placeholders filled in after pass A (pools opened late to keep
        # pass-A SBUF pressure down)
        hp2 = crp = None
        hr_a = [[None] * NCT for _ in range(n_tiles2)]
        gr_a = [[None] * NCT for _ in range(n_tiles2)]
        cr_a = [[None] * NCT for _ in range(n_tiles2)]
        x1_a = [[None] * NCT for _ in range(n_tiles2)]
        ac_a = [[None] * NCT for _ in range(n_tiles2)]
        u_0 = [None] * NCT
        v_0 = [None] * NCT

        def p2_loads(t):
            # hr/gr ride the sync queue (consumers live on gpsimd and
            # vector), cr/x1 the gpsimd queue (consumers on scalar and
            # vector): no issue stream ever waits on work queued behind
            # it on its own engine
            for i in range(NCT):
                hri = hp2.tile([NP, L2], BF16, name=f"hr_{t}_{i}", tag="hr")
                nc.sync.dma_start(
                    hri[:],
                    h_d.ap()[i * NP:(i + 1) * NP, t * L2:(t + 1) * L2])
                hr_a[t][i] = hri
                gri = hp2.tile([NP, L2], BF16, name=f"gr_{t}_{i}", tag="gr")
                nc.sync.dma_start(
                    gri[:],
                    g_d.ap()[i * NP:(i + 1) * NP, t * L2:(t + 1) * L2])
                gr_a[t][i] = gri
                cri = crp.tile([NP, L2], BF16, name=f"cr_{t}_{i}", tag="cr",
                               bufs=7)
                nc.gpsimd.dma_start(
                    cri[:],
                    cs_d.ap()[i * NP:(i + 1) * NP, t * L2:(t + 1) * L2])
                cr_a[t][i] = cri
                # x1 tile starts as the bf16 residual x; the output
                # projection accumulates into it in place
                x1i = crp.tile([NP, L2], BF16, name=f"x1_{t}_{i}",
                               tag="x1", bufs=10)
                nc.gpsimd.dma_start(
                    x1i[:],
                    x_d.ap()[i * NP:(i + 1) * NP,
                             3 + t * L2:3 + (t + 1) * L2])
                x1_a[t][i] = x1i

        # =========================== PHASE 1 ===========================
        with ExitStack() as p1:
            w1b = p1.enter_context(tc.tile_pool(name="w1b", bufs=1))
            wg_sb = w1b.tile([NP, NCT * H2], BF16, name="wg_sb")
            # zcb stays resident between pass A and pass B
            zres = p1.enter_context(tc.tile_pool(name="zres", bufs=1))
            zcb_res = [zres.tile([NP, NCT * L], BF16, name=f"zcbr_{t}")
                       for t in range(n_tiles)]

            # ------------------------- PASS A -------------------------
            with ExitStack() as pA:
                w1a = pA.enter_context(tc.tile_pool(name="w1a", bufs=1))
                wi_sb = w1a.tile([NP, NCT * H2], BF16, name="wi_sb")
                # z-half first (needed by the halo projection); x tile 0
                # loads on sync in parallel
                for k in range(NCT):
                    [nc.scalar, nc.gpsimd][k % 2].dma_start(
                        wi_sb[:, k * H2 + D:(k + 1) * H2],
                        wi_d.ap()[k * NP:(k + 1) * NP, D:])
                for k in range(NCT):
                    [nc.scalar, nc.gpsimd][k % 2].dma_start(
                        wi_sb[:, k * H2:k * H2 + D],
                        wi_d.ap()[k * NP:(k + 1) * NP, :D])
                nc.scalar.dma_start(blk_sb(wg_sb[:], NCT),
                                    blk_in(wg_d.ap()[:, :], NCT))

                pmm = pA.enter_context(
                    tc.tile_pool(name="pmm", bufs=6, space="PSUM"))
                xbp = pA.enter_context(tc.tile_pool(name="xbp", bufs=2))
                sqp = pA.enter_context(tc.tile_pool(name="sqp", bufs=2))
                sp = pA.enter_context(tc.tile_pool(name="sp", bufs=2))
                zp = pA.enter_context(tc.tile_pool(name="zp", bufs=9))
                zcp = pA.enter_context(tc.tile_pool(name="zcp", bufs=4))
                gp = pA.enter_context(tc.tile_pool(name="gp", bufs=3))
                gbp = pA.enter_context(tc.tile_pool(name="gbp", bufs=2))
                zhp = pA.enter_context(tc.tile_pool(name="zhp", bufs=1))

                # ---- halo: z for the 3 tokens before this chunk ----
                xbh = xbp.tile([NP, 3 * NCT], BF16, name="xbh", tag="xbh",
                               bufs=1)
                nc.sync.dma_start(blk_sb(xbh[:], NCT),
                                  blk_in(x_d.ap()[:, 0:3], NCT))
                xsqh = sqp.tile([NP, 3 * NCT], BF16, name="xsqh",
                                tag="xsqh", bufs=1)
                nc.vector.tensor_tensor(xsqh[:], xbh[:], xbh[:], OP.mult)
                ssqh = pmm.tile([NP, 3], F32, name="ssqh", tag="mm")
                for i in range(NCT):
                    nc.tensor.matmul(ssqh[:], ones_bf[:],
                                     xsqh[:, 3 * i:3 * i + 3],
                                     start=(i == 0), stop=(i == NCT - 1))
                shq = sp.tile([NP, 3], F32, name="shq", tag="sq")
                nc.scalar.activation(shq[:], ssqh[:], AF.Sqrt,
                                     bias=epsb[:, 0:1])
                sh_ = sp.tile([NP, 3], F32, name="sh_", tag="s")
                nc.vector.reciprocal(sh_[:], shq[:])
                zhalo = zhp.tile([NP, 3 * NCT], BF16, name="zhalo",
                                 tag="zhalo")
                for m in range(NCT):
                    ps = pmm.tile([NP, 3], F32, name=f"zh_ps_{m}", tag="mm")
                    for k in range(NCT):
                        lhs = wi_sb[:, k * H2 + D + m * NP:
                                    k * H2 + D + (m + 1) * NP]
                        nc.tensor.matmul(ps[:], lhs,
                                         xbh[:, 3 * k:3 * k + 3],
                                         start=(k == 0), stop=(k == NCT - 1))
                    nc.vector.tensor_tensor(zhalo[:, 3 * m:3 * m + 3], ps[:],
                                            sh_[:], OP.mult)

                z_prev = [None] * NCT
                xb_t = [None, None]
                xb_t[0] = xbp.tile([NP, NCT * L], BF16, name="xb_0",
                                   tag="xb")
                nc.sync.dma_start(blk_sb(xb_t[0][:], NCT),
                                  blk_in(x_d.ap()[:, 3:3 + L], NCT))
                for t in range(n_tiles):
                    xb = xb_t[t % 2]
                    if t + 1 < n_tiles:
                        c0 = 3 + (t + 1) * L
                        xb_t[(t + 1) % 2] = xbp.tile(
                            [NP, NCT * L], BF16, name=f"xb_{t + 1}",
                            tag="xb")
                        nc.sync.dma_start(
                            blk_sb(xb_t[(t + 1) % 2][:], NCT),
                            blk_in(x_d.ap()[:, c0:c0 + L], NCT))
                    if t == 1:  # wo prefetch once the wi queue has drained
                        nc.sync.dma_start(blk_sb(wo_sb[:], NCT),
                                          blk_in(wo_d.ap()[:, :], NCT))

                    # token norm scale s = 1/||x|| (broadcast over parts)
                    xsq = sqp.tile([NP, NCT * L], BF16, name=f"xsq_{t}",
                                   tag="xsq")
                    nc.scalar.activation(xsq[:], xb[:], AF.Square)
                    ssq = pmm.tile([NP, L], F32, name=f"ssq_{t}", tag="mm")
                    for i in range(NCT):
                        nc.tensor.matmul(ssq[:], ones_bf[:],
                                         xsq[:, i * L:(i + 1) * L],
                                         start=(i == 0), stop=(i == NCT - 1))
                    sq_ = sp.tile([NP, L], F32, name=f"sq_{t}", tag="sq")
                    nc.scalar.activation(sq_[:], ssq[:], AF.Sqrt,
                                         bias=epsb[:, 0:1])
                    s = sp.tile([NP, L], F32, name=f"s_{t}", tag="s")
                    nc.vector.reciprocal(s[:], sq_[:])

                    # input proj, z half
                    z_cur = [None] * NCT
                    for m in range(NCT):
                        ps = pmm.tile([NP, L], F32, name=f"u_ps_{t}_{m}",
                                      tag="mm")
                        for k in range(NCT):
                            lhs = wi_sb[:, k * H2 + D + m * NP:
                                        k * H2 + D + (m + 1) * NP]
                            nc.tensor.matmul(ps[:], lhs,
                                             xb[:, k * L:(k + 1) * L],
                                             start=(k == 0),
                                             stop=(k == NCT - 1))
                        zt = zp.tile([NP, L + 3], BF16, name=f"z_{t}_{m}",
                                     tag="z")
                        nc.vector.tensor_tensor(zt[:, 3:3 + L], ps[:], s[:],
                                                OP.mult)
                        if t == 0:
                            nc.gpsimd.tensor_copy(zt[:, 0:3],
                                                  zhalo[:, 3 * m:3 * m + 3])
                        else:
                            nc.gpsimd.tensor_copy(zt[:, 0:3],
                                                  z_prev[m][:, L:L + 3])
                        z_cur[m] = zt
                    z_prev = z_cur

                    # depthwise causal conv (taps 0-1 vector, 2-3 gpsimd)
                    for i in range(NCT):
                        zci = zcp.tile([NP, L], BF16, name=f"zc_{t}_{i}",
                                       tag="zc")
                        nc.vector.tensor_scalar(
                            zci[:], z_cur[i][:, 0:L],
                            cw_sb[:, 0 * NCT + i:0 * NCT + i + 1],
                            cb_sb[:, i:i + 1], op0=OP.mult, op1=OP.add)
                        nc.vector.scalar_tensor_tensor(
                            zci[:], z_cur[i][:, 1:1 + L],
                            cw_sb[:, 1 * NCT + i:1 * NCT + i + 1],
                            zci[:], op0=OP.mult, op1=OP.add)
                        nc.vector.scalar_tensor_tensor(
                            zci[:], z_cur[i][:, 2:2 + L],
                            cw_sb[:, 2 * NCT + i:2 * NCT + i + 1],
                            zci[:], op0=OP.mult, op1=OP.add)
                        nc.vector.scalar_tensor_tensor(
                            zcb_res[t][:, i * L:(i + 1) * L],
                            z_cur[i][:, 3:3 + L],
                            cw_sb[:, 3 * NCT + i:3 * NCT + i + 1],
                            zci[:], op0=OP.mult, op1=OP.add)

                    # gate half of the input proj -> gelu -> spill
                    gbig = gbp.tile([NP, NCT * L], BF16, name=f"gbig_{t}",
                                    tag="gbig")
                    for m in range(NCT):
                        ps = pmm.tile([NP, L], F32, name=f"g_ps_{t}_{m}",
                                      tag="mm")
                        for k in range(NCT):
                            lhs = wi_sb[:, k * H2 + m * NP:
                                        k * H2 + (m + 1) * NP]
                            nc.tensor.matmul(ps[:], lhs,
                                             xb[:, k * L:(k + 1) * L],
                                             start=(k == 0),
                                             stop=(k == NCT - 1))
                        g1 = gp.tile([NP, L], BF16, name=f"g1_{t}_{m}",
                                     tag="g1")
                        nc.vector.tensor_tensor(g1[:], ps[:], s[:], OP.mult)
                        if gelu_approx:
                            gs = gp.tile([NP, L], F32, name=f"gs_{t}_{m}",
                                         tag="gs", bufs=2)
                            nc.scalar.activation(gs[:], g1[:], AF.Sigmoid,
                                                 scale=1.702)
                            nc.vector.tensor_tensor(
                                gbig[:, m * L:(m + 1) * L], g1[:], gs[:],
                                OP.mult)
                        else:
                            nc.scalar.activation(gbig[:, m * L:(m + 1) * L],
                                                 g1[:], GELU)
                    nc.sync.dma_start(
                        blk_in(g_d.ap()[:, t * L:(t + 1) * L], NCT),
                        blk_sb(gbig[:], NCT))

            # ------------------- PASS B (wi freed) --------------------
            hp2 = top.enter_context(tc.tile_pool(name="hp2", bufs=7,
                                                 side="right"))
            crp = top.enter_context(tc.tile_pool(name="crp", bufs=8,
                                                 side="right"))

            with ExitStack() as pB:
                pmmb = pB.enter_context(
                    tc.tile_pool(name="pmmb", bufs=6, space="PSUM"))
                sfp = pB.enter_context(tc.tile_pool(name="sfp", bufs=8))
                sip = pB.enter_context(tc.tile_pool(name="sip", bufs=8))
                ap_ = pB.enter_context(tc.tile_pool(name="ap", bufs=3))
                a2p = pB.enter_context(tc.tile_pool(name="a2p", bufs=2))
                bp = pB.enter_context(tc.tile_pool(name="bp", bufs=2))
                xsp = pB.enter_context(tc.tile_pool(name="xsp", bufs=2))
                lap = pB.enter_context(tc.tile_pool(name="lap", bufs=2))
                hbp = pB.enter_context(tc.tile_pool(name="hbp", bufs=1))
                csp = pB.enter_context(tc.tile_pool(name="csp", bufs=1))
                # per-channel -8*softplus(fb) broadcast along tokens (lets
                # the la multiply run on gpsimd, which cannot take scalar
                # pointers)
                mspp = pB.enter_context(tc.tile_pool(name="mspp", bufs=1))
                mspb = mspp.tile([NP, NCT * L], BF16, name="mspb")
                for i in range(NCT):
                    nc.vector.tensor_scalar(mspb[:, i * L:(i + 1) * L],
                                            ones_f[:, 0:L],
                                            msp_sb[:, i:i + 1],
                                            None, op0=OP.mult)
                for t in range(n_tiles):
                    zcb = zcb_res[t]
                    sf = [None] * NCT
                    si = [None] * NCT
                    for i in range(NCT):
                        psf = pmmb.tile([NP, L], F32, name=f"f_ps_{t}_{i}",
                                        tag="mmb")
                        for k in range(NCT):
                            lhs = wg_sb[:, k * H2 + i * NP:
                                        k * H2 + (i + 1) * NP]
                            nc.tensor.matmul(psf[:], lhs,
                                             zcb[:, k * L:(k + 1) * L],
                                             start=(k == 0),
                                             stop=(k == NCT - 1))
                        sfi = sfp.tile([NP, L], BF16, name=f"sf_{t}_{i}",
                                       tag="sf")
                        nc.scalar.activation(sfi[:], psf[:], AF.Sigmoid,
                                             bias=gbf_sb[:, i:i + 1])
                        sf[i] = sfi
                        psi = pmmb.tile([NP, L], F32, name=f"i_ps_{t}_{i}",
                                        tag="mmb")
                        for k in range(NCT):
                            lhs = wg_sb[:, k * H2 + D + i * NP:
                                        k * H2 + D + (i + 1) * NP]
                            nc.tensor.matmul(psi[:], lhs,
                                             zcb[:, k * L:(k + 1) * L],
                                             start=(k == 0),
                                             stop=(k == NCT - 1))
                        sii = sip.tile([NP, L], BF16, name=f"si_{t}_{i}",
                                       tag="si")
                        nc.scalar.activation(sii[:], psi[:], AF.Sigmoid,
                                             bias=gbi_sb[:, i:i + 1])
                        si[i] = sii

                    alpha = [None] * NCT
                    for i in range(NCT):
                        al = ap_.tile([NP, L], F32, name=f"al_{t}_{i}",
                                      tag="alpha")
                        nc.scalar.activation(al[:], sf[i][:], AF.Exp,
                                             scale=msp_sb[:, i:i + 1])
                        alpha[i] = al
                    beta = [None] * NCT
                    for i in range(NCT):
                        # a2 = alpha^2 <= 1 exactly: 1+1e-6-a2 > 0 always
                        a2i = a2p.tile([NP, L], F32, name=f"a2_{t}_{i}",
                                       tag="a2")
                        nc.gpsimd.tensor_tensor(a2i[:], alpha[i][:],
                                                alpha[i][:], OP.mult)
                        be = bp.tile([NP, L], BF16, name=f"be_{t}_{i}",
                                     tag="beta")
                        nc.scalar.activation(be[:], a2i[:], AF.Sqrt,
                                             scale=-1.0, bias=onepb[:, 0:1])
                        beta[i] = be

                    hbig = hbp.tile([NP, NCT * L], BF16, name=f"hbig_{t}",
                                    tag="hbig")
                    csbig = csp.tile([NP, NCT * L], BF16, name=f"csbig_{t}",
                                     tag="csbig")
                    # h scans first: they carry the cross-tile recurrence
                    # and (at t=3) gate the carry exchange
                    for i in range(NCT):
                        xs = xsp.tile([NP, L], BF16, name=f"xs_{t}_{i}",
                                      tag="xs")
                        nc.gpsimd.tensor_tensor(xs[:], si[i][:],
                                                zcb[:, i * L:(i + 1) * L],
                                                OP.mult)
                        nc.vector.tensor_tensor(xs[:], xs[:], beta[i][:],
                                                OP.mult)
                        h_init = 0.0 if t == 0 else hlast[:, i:i + 1]
                        nc.vector.tensor_tensor_scan(
                            hbig[:, i * L:(i + 1) * L], alpha[i][:], xs[:],
                            h_init, op0=OP.mult, op1=OP.add)
                        if t < n_tiles - 1:
                            nc.gpsimd.tensor_copy(
                                hlast[:, i:i + 1],
                                hbig[:, (i + 1) * L - 1:(i + 1) * L])
                    if t == n_tiles - 1:
                        # last-token h of every channel in one strided DMA
                        with tc.high_priority():
                            nc.sync.dma_start(
                                carry_loc.ap()[0:1, :].rearrange(
                                    "a (i p) -> p i a", i=NCT),
                                hbig[:, L - 1:NCT * L:L])
                    nc.sync.dma_start(
                        blk_in(h_d.ap()[:, t * L:(t + 1) * L], NCT),
                        blk_sb(hbig[:], NCT))
                    for i in range(NCT):
                        la = lap.tile([NP, L], BF16, name=f"la_{t}_{i}",
                                      tag="la")
                        nc.gpsimd.tensor_tensor(la[:], sf[i][:],
                                                mspb[:, i * L:(i + 1) * L],
                                                OP.mult)
                        c_init = 0.0 if t == 0 else cslast[:, i:i + 1]
                        nc.vector.tensor_tensor_scan(
                            csbig[:, i * L:(i + 1) * L], ones_f[:, 0:L],
                            la[:], c_init, op0=OP.mult, op1=OP.add)
                        if t < n_tiles - 1:
                            nc.gpsimd.tensor_copy(
                                cslast[:, i:i + 1],
                                csbig[:, (i + 1) * L - 1:(i + 1) * L])
                    nc.sync.dma_start(
                        blk_in(cs_d.ap()[:, t * L:(t + 1) * L], NCT),
                        blk_sb(csbig[:], NCT))

                    # phase-2 head (tile 0) rides the pass-B tail
                    if t == 1:
                        p2_loads(0)
                    elif t == 2:
                        for i in range(NCT):
                            aci = crp.tile([NP, L2], BF16, name=f"ac_0_{i}",
                                           tag="ac")
                            nc.scalar.activation(aci[:], cr_a[0][i][:],
                                                 AF.Exp)
                            ac_a[0][i] = aci
                    elif t == 3:
                        for i in range(NCT):
                            ui = crp.tile([NP, L2], BF16, name=f"u_0_{i}",
                                          tag="gh", bufs=8)
                            nc.gpsimd.tensor_tensor(ui[:], gr_a[0][i][:],
                                                    hr_a[0][i][:], OP.mult)
                            u_0[i] = ui
                            vi = crp.tile([NP, L2], BF16, name=f"v_0_{i}",
                                          tag="v")
                            nc.gpsimd.tensor_tensor(vi[:], gr_a[0][i][:],
                                                    ac_a[0][i][:], OP.mult)
                            v_0[i] = vi

                # ---- pairwise carry exchange ----
                with tc.high_priority():
                    nc.gpsimd.collective_compute(
                        "AllGather", OP.bypass,
                        replica_groups=[[0, 1], [2, 3], [4, 5], [6, 7]],
                        ins=[carry_loc.ap()], outs=[carry_gth.ap()])

        # =========================== PHASE 2 ===========================
        # stream the gmlp weights now that wi/wg/zcb have freed
        w2c = top.enter_context(tc.tile_pool(name="w2c", bufs=1,
                                             side="right"))
        wgr_lo = w2c.tile([NP, (NCT // 2) * HID], BF16, name="wgr_lo")
        wgr_hi = w2c.tile([NP, (NCT // 2) * HID], BF16, name="wgr_hi")
        wsh_sb = w2c.tile([NP, 2 * NCT * D], BF16, name="wsh_sb")
        wq = [nc.scalar, nc.sync]
        for k in range(NCT // 2):
            wq[k % 2].dma_start(wgr_lo[:, k * HID:(k + 1) * HID],
                                wgr_d.ap()[k * NP:(k + 1) * NP, :])
        for k in range(NCT // 2):
            wq[k % 2].dma_start(
                wgr_hi[:, k * HID:(k + 1) * HID],
                wgr_d.ap()[(k + NCT // 2) * NP:(k + NCT // 2 + 1) * NP, :])
        nc.scalar.dma_start(blk_sb(wsh_sb[:, 0:NCT * D], NCT),
                            blk_in(wsh_d.ap()[0:D, :], NCT))
        nc.sync.dma_start(blk_sb(wsh_sb[:, NCT * D:], NCT),
                            blk_in(wsh_d.ap()[D:, :], NCT))

        with ExitStack() as p2:
            hfp = p2.enter_context(tc.tile_pool(name="hfp", bufs=2))
            sq2p = p2.enter_context(tc.tile_pool(name="sq2p", bufs=1))
            s2p = p2.enter_context(tc.tile_pool(name="s2p", bufs=2))
            t2p = p2.enter_context(tc.tile_pool(name="t2p", bufs=2))
            t2gp = p2.enter_context(tc.tile_pool(name="t2gp", bufs=2))
            gvp = p2.enter_context(tc.tile_pool(name="gvp", bufs=16))
            op_ = p2.enter_context(tc.tile_pool(name="op", bufs=2))
            pgro = p2.enter_context(
                tc.tile_pool(name="pgro", bufs=5, space="PSUM"))
            pssq2 = p2.enter_context(
                tc.tile_pool(name="pssq2", bufs=1, space="PSUM"))

            p2_loads(1)

            cg = cpool.tile([NP, NCT], BF16, name="cg")
            nc.gpsimd.dma_start(
                cg[:],
                carry_gth.ap()[0:1, :].rearrange("a (i p) -> p (i a)",
                                                 i=NCT))

            # carry-free half Wo@(g*h) for tile 0 flows straight out
            # of pass B and hides the collective latency
            with tc.high_priority():
                for t, u_t in ((0, u_0),):
                    x1 = x1_a[t]
                    for m in range(NCT):
                        ps = pmm2.tile([NP, L2], F32, name=f"u_ps_{t}_{m}",
                                       tag="mm2")
                        for k in range(NCT):
                            lhs = wo_sb[:, k * D + m * NP:
                                        k * D + (m + 1) * NP]
                            nc.tensor.matmul(ps[:], lhs, u_t[k][:],
                                             start=(k == 0),
                                             stop=(k == NCT - 1))
                        nc.vector.tensor_tensor(x1[m][:], ps[:], x1[m][:],
                                                OP.add)

            # the carry path must not be scheduled ahead of the
            # carry-free projections: the static scheduler models the
            # collective as fast, so hold these back explicitly
            with tc.tile_wait_until(0.40):
                cmd = cpool.tile([NP, 1], F32, name="cmd")
                nc.vector.tensor_tensor(cmd[:], cmask_sb[:, 0:1],
                                        x1_a[0][NCT - 1][:, 0:1], OP.bypass)
                carrym = cpool.tile([NP, NCT], F32, name="carrym")
                nc.vector.tensor_scalar(carrym[:], cg[:], cmd[:, 0:1], None,
                                        op0=OP.mult)

                for t, v_t in ((0, v_0),):
                    x1 = x1_a[t]
                    for i in range(NCT):
                        nc.vector.tensor_scalar(v_t[i][:], v_t[i][:],
                                                carrym[:, i:i + 1], None,
                                                op0=OP.mult)
                    for m in range(NCT):
                        ps = pmm2.tile([NP, L2], F32,
                                       name=f"v_ps2_{t}_{m}", tag="mm2")
                        for k in range(NCT):
                            lhs = wo_sb[:, k * D + m * NP:
                                        k * D + (m + 1) * NP]
                            nc.tensor.matmul(ps[:], lhs, v_t[k][:],
                                             start=(k == 0),
                                             stop=(k == NCT - 1))
                        nc.vector.tensor_tensor(x1[m][:], ps[:], x1[m][:],
                                                OP.add)

            for t in range(n_tiles2):
                hr, gr, cr, x1 = hr_a[t], gr_a[t], cr_a[t], x1_a[t]
                if t >= 1:
                    acs = [None] * NCT
                    for i in range(NCT):
                        aci = crp.tile([NP, L2], BF16, name=f"ac_{t}_{i}",
                                       tag="ac")
                        nc.scalar.activation(aci[:], cr[i][:], AF.Exp)
                        acs[i] = aci
                    gh = [None] * NCT
                    for i in range(NCT):
                        hfi = hfp.tile([NP, L2], BF16, name=f"hf_{t}_{i}",
                                       tag="hf")
                        nc.vector.scalar_tensor_tensor(hfi[:], acs[i][:],
                                                       carrym[:, i:i + 1],
                                                       hr[i][:],
                                                       op0=OP.mult,
                                                       op1=OP.add)
                        ghi = crp.tile([NP, L2], BF16, name=f"gh_{t}_{i}",
                                       tag="gh", bufs=8)
                        nc.vector.tensor_tensor(ghi[:], gr[i][:], hfi[:],
                                                OP.mult)
                        gh[i] = ghi
                    for m in range(NCT):
                        ps = pmm2.tile([NP, L2], F32, name=f"o_ps_{t}_{m}",
                                       tag="mm2")
                        for k in range(NCT):
                            lhs = wo_sb[:, k * D + m * NP:
                                        k * D + (m + 1) * NP]
                            nc.tensor.matmul(ps[:], lhs, gh[k][:],
                                             start=(k == 0),
                                             stop=(k == NCT - 1))
                        nc.vector.tensor_tensor(x1[m][:], ps[:], x1[m][:],
                                                OP.add)

                # prefetch reloads for tile t+2 (t+1 already in flight)
                if t + 2 < n_tiles2:
                    p2_loads(t + 2)

                # x1 norm scale (broadcast), applied at drains
                ssq2 = pssq2.tile([NP, L2], F32, name=f"ssq2_{t}",
                                  tag="ssq2")
                for i in range(NCT):
                    xsq = sq2p.tile([NP, L2], BF16, name=f"x1sq_{t}_{i}",
                                    tag="x1sq")
                    nc.vector.tensor_tensor(xsq[:], x1[i][:], x1[i][:],
                                            OP.mult)
                    nc.tensor.matmul(ssq2[:], ones_bf[:], xsq[:],
                                     start=(i == 0), stop=(i == NCT - 1))
                s2q = s2p.tile([NP, L2], F32, name=f"s2q_{t}", tag="s2q",
                               bufs=1)
                nc.scalar.activation(s2q[:], ssq2[:], AF.Sqrt,
                                     bias=epsb[:, 0:1])
                s2 = s2p.tile([NP, L2], F32, name=f"s2_{t}", tag="s2")
                nc.vector.reciprocal(s2[:], s2q[:])

                # grow proj: gate2 rows [0:2D), v rows [2D:4D)
                gv = [None] * (2 * NCT)
                for hm in range(2 * NCT):
                    psg = pgro.tile([NP, L2], F32, name=f"g2_ps_{t}_{hm}",
                                    tag="mm2g")
                    for k in range(NCT):
                        wsb = wgr_lo if k < NCT // 2 else wgr_hi
                        kk = k % (NCT // 2)
                        lhs = wsb[:, kk * HID + hm * NP:
                                  kk * HID + (hm + 1) * NP]
                        nc.tensor.matmul(psg[:], lhs, x1[k][:],
                                         start=(k == 0), stop=(k == NCT - 1))
                    psv = pgro.tile([NP, L2], F32, name=f"v_ps_{t}_{hm}",
                                    tag="mm2g")
                    for k in range(NCT):
                        wsb = wgr_lo if k < NCT // 2 else wgr_hi
                        kk = k % (NCT // 2)
                        lhs = wsb[:, kk * HID + H2 + hm * NP:
                                  kk * HID + H2 + (hm + 1) * NP]
                        nc.tensor.matmul(psv[:], lhs, x1[k][:],
                                         start=(k == 0), stop=(k == NCT - 1))
                    t2 = t2p.tile([NP, L2], BF16, name=f"t2_{t}_{hm}",
                                  tag="t2")
                    nc.vector.tensor_tensor(t2[:], psg[:], s2[:], OP.mult)
                    t2g = t2gp.tile([NP, L2], BF16, name=f"t2g_{t}_{hm}",
                                    tag="t2g")
                    if gelu_approx:
                        sg2 = t2gp.tile([NP, L2], F32, name=f"sg2_{t}_{hm}",
                                        tag="sg2")
                        nc.scalar.activation(sg2[:], t2[:], AF.Sigmoid,
                                             scale=1.702)
                        nc.vector.tensor_tensor(t2g[:], t2[:], sg2[:],
                                                OP.mult)
                    else:
                        nc.scalar.activation(t2g[:], t2[:], GELU)
                    gvi = gvp.tile([NP, L2], BF16, name=f"gv_{t}_{hm}",
                                   tag="gv")
                    nc.vector.tensor_tensor(gvi[:], t2g[:], psv[:], OP.mult)
                    gv[hm] = gvi

                # shrink proj (x s2) + residual -> out
                for m in range(NCT):
                    ps = pmm2.tile([NP, L2], F32, name=f"s_ps_{t}_{m}",
                                   tag="mm2")
                    for k in range(2 * NCT):
                        lhs = wsh_sb[:, k * D + m * NP: k * D + (m + 1) * NP]
                        nc.tensor.matmul(ps[:], lhs, gv[k][:],
                                         start=(k == 0),
                                         stop=(k == 2 * NCT - 1))
                    om = op_.tile([NP, L2], F32, name=f"out_{t}_{m}",
                                  tag="out")
                    nc.vector.tensor_tensor(om[:], ps[:], s2[:], OP.mult)
                    nc.vector.tensor_tensor(om[:], om[:], x1[m][:], OP.add)
                    nc.sync.dma_start(
                        out_d.ap()[m * NP:(m + 1) * NP, t * L2:(t + 1) * L2],
                        om[:])

    nc.compile()
    return nc


def host_prepare(inputs, T_core, n_cores=N_CORES):
    """Build per-core in_maps from full inputs."""
    x = np.asarray(inputs["x"], np.float32)            # [B, T, D]
    B, T, _ = x.shape
    halves = n_cores // B
    assert T == halves * T_core

    gam1 = np.asarray(inputs["hawk_norm_gamma"], np.float32)
    gam2 = np.asarray(inputs["gmlp_norm_gamma"], np.float32)
    scale1 = gam1 * np.sqrt(D)
    scale2 = gam2 * np.sqrt(D)

    wi = (np.asarray(inputs["input_w"], np.float32) * scale1[None, :]).T
    wg = np.asarray(inputs["gates_w"], np.float32).T
    wo = np.asarray(inputs["output_w"], np.float32).T
    wgr = (np.asarray(inputs["grow_w"], np.float32) * scale2[None, :]).T
    wsh = np.asarray(inputs["shrink_w"], np.float32).T

    fb = np.asarray(inputs["forget_base"], np.float64)
    msp = (-8.0 * np.log1p(np.exp(fb))).astype(np.float32)

    def chan_layout(v):  # [D] -> [128, 8] with [p, i] = v[128*i + p]
        return np.ascontiguousarray(v.reshape(NCT, NP).T)

    gb = np.asarray(inputs["gates_b"], np.float32)
    cw = np.asarray(inputs["conv_w"], np.float32)[:, 0, :]   # [D, K]
    cb = np.asarray(inputs["conv_b"], np.float32)

    shared = {
        "wi": wi.astype(_BF), "wg": wg.astype(_BF), "wo": wo.astype(_BF),
        "wgr": wgr.astype(_BF), "wsh": wsh.astype(_BF),
        "msp": chan_layout(msp),
        "gbf": chan_layout(gb[:D]), "gbi": chan_layout(gb[D:]),
        "cw": np.concatenate([chan_layout(cw[:, k]) for k in range(KCONV)],
                             axis=1),
        "cb": chan_layout(cb),
    }
    in_maps = []
    for core in range(n_cores):
        b, h = core // halves, core % halves
        xf = np.zeros((D, 3 + T_core), np.float32)
        xf[:, 3:] = x[b, h * T_core:(h + 1) * T_core, :].T
        if h > 0:
            xf[:, 0:3] = x[b, h * T_core - 3:h * T_core, :].T
        m = dict(shared)
        m["x"] = xf.astype(_BF)
        m["cmask"] = np.full((NP, 1), 1.0 if h > 0 else 0.0, np.float32)
        in_maps.append(m)
    return in_maps


def assemble_output(results, B, T, T_core, n_cores=N_CORES):
    halves = n_cores // B
    out = np.empty((B, T, D), np.float32)
    for core in range(n_cores):
        b, h = core // halves, core % halves
        out[b, h * T_core:(h + 1) * T_core, :] = results[core]["out"].T
    return out


_PROG_CACHE = {}


def kernel(**inputs) -> np.ndarray:
    x = np.asarray(inputs["x"])
    B, T, _ = x.shape
    T_core = T * B // N_CORES
    L = 512 if T_core % 512 == 0 else T_core // 4
    key = (T_core, L)
    if key not in _PROG_CACHE:
        _PROG_CACHE[key] = build_program(T_core, L)
    nc = _PROG_CACHE[key]
    in_maps = host_prepare(inputs, T_core)
    res = run_bass_kernel_spmd(nc, in_maps, list(range(N_CORES)))
    return assemble_output(res.results, B, T, T_core)
